# revision 27
# baseline (speedup 1.0000x reference)
"""Adaptive-attention Bass kernel for 8 TRN2 NeuronCores (v2).

Data-parallel over the B = b*w = 512 pseudo-batch: core k handles
B in [64k, 64k+64)  (image b = k//2, w-slice of 64 columns).

Per-core pipeline (bf16 compute, f32 PSUM accumulation):
  A) value conv (PE GEMM) -> v tiles [j | (n,d)] spilled to DRAM; vm/V
  B) per group-pair (2 x 16 B packed on partitions 0-47 / 64-111):
     conv1 (PE, 3 shifted matmuls, block-diag weights) -> relu (ACT)
     -> depthwise conv2 (DVE, flat aligned bf16 scalar_tensor_tensor)
     -> relu (ACT) -> A replicated over heads (gpsimd SWDGE)
     -> involution (DVE stt) -> attn_new (padded rows, ACT DMA ring)
  C) attn_new read back natural, PE-transpose via identity, attn@v (PE),
     out conv (PE GEMM) -> out

Host pads attn rows to 130 cols (zero guards) so loads/stores are fully
contiguous; attn_new returns padded and is stripped on host.
"""

import sys

sys.path.insert(0, "/opt/trn_rl_repo")

from contextlib import ExitStack

import ml_dtypes
import numpy as np

import concourse.bacc as bacc
import concourse.bass as bass
import concourse.tile as tile
from concourse import mybir
from concourse.bass_utils import run_bass_kernel_spmd

BF = mybir.dt.bfloat16
F32 = mybir.dt.float32
NPBF = ml_dtypes.bfloat16

NCORES = 8
BC = 64
HEADS = 8
P = 128
D = 64
C = 512
K = 3
NG = 4
GB = 16
NPAIR = 2
NCH = 4
IC = 32
JP = 130
ROWS = 36
FL = IC * JP  # 4160

mult = mybir.AluOpType.mult
add = mybir.AluOpType.add
Relu = mybir.ActivationFunctionType.Relu
Copy = mybir.ActivationFunctionType.Copy


def build_nc(use_bv: bool, use_bo: bool) -> bass.Bass:
    nc = bacc.Bacc(target_bir_lowering=False)

    attn_d = nc.dram_tensor("attn_in", [BC * HEADS, P * JP], BF, kind="ExternalInput")
    xT_d = nc.dram_tensor("xT", [C, BC * P], BF, kind="ExternalInput")
    wvT_d = nc.dram_tensor("wvT", [C, C], BF, kind="ExternalInput")
    woT_d = nc.dram_tensor("woT", [C, C], BF, kind="ExternalInput")
    w1bd_d = nc.dram_tensor("w1bd", [K, 128, GB * K], BF, kind="ExternalInput")
    w2s_d = nc.dram_tensor("w2s", [128, 9], F32, kind="ExternalInput")
    b1c_d = nc.dram_tensor("b1c", [128, 1], F32, kind="ExternalInput")
    b2c_d = nc.dram_tensor("b2c", [128, 1], F32, kind="ExternalInput")
    wlbl_d = nc.dram_tensor("wlbl", [1, 2 * K], F32, kind="ExternalInput")
    iden_d = nc.dram_tensor("iden", [128, 128], BF, kind="ExternalInput")
    wvs_d = nc.dram_tensor("wvs", [C, HEADS], BF, kind="ExternalInput")
    if use_bv:
        bvr_d = nc.dram_tensor("bvr", [1, C], BF, kind="ExternalInput")
        bvs_d = nc.dram_tensor("bvs", [1, HEADS], BF, kind="ExternalInput")
    if use_bo:
        bor_d = nc.dram_tensor("bor", [1, C], BF, kind="ExternalInput")

    with tile.TileContext(nc) as tc, ExitStack() as ctx:
        pdram = ctx.enter_context(tc.tile_pool(name="dram", bufs=1, space="DRAM"))
        anew_p = [pdram.tile([BC * HEADS // 2, P * JP], BF, kind="ExternalOutput",
                             name=f"attn_new{i}", uniquify=False)
                  for i in range(2)]
        outT_t = pdram.tile([BC, P, C], BF, kind="ExternalOutput",
                            name="outT", uniquify=False)
        vdram = pdram.tile([BC, 128, C], BF, name="vdram")

        const = ctx.enter_context(tc.tile_pool(name="const", bufs=1))
        wvT_sb = const.tile([128, 4, C], BF)
        woT_sb = const.tile([128, 4, C], BF)
        w1bd_sb = const.tile([128, K, GB * K], BF)
        w2s_sb = const.tile([128, 9], F32)
        b1c_sb = const.tile([128, 1], F32)
        b2c_sb = const.tile([128, 1], F32)
        wlbl_sb = const.tile([1, 2 * K], F32)
        iden_sb = const.tile([128, 128], BF)
        wvs_sb = const.tile([128, 4, HEADS], BF)
        ones_col = const.tile([128, 1], F32)
        ones_row = const.tile([1, 128], BF)
        vrows = const.tile([1, K, C], F32)
        vcols = const.tile([128, NG, K], F32)
        vpart = const.tile([128, BC, HEADS], F32)

        for kk in range(4):
            nc.sync.dma_start(out=wvT_sb[:, kk, :], in_=wvT_d[kk * 128:(kk + 1) * 128, :])
            nc.sync.dma_start(out=woT_sb[:, kk, :], in_=woT_d[kk * 128:(kk + 1) * 128, :])
        for t in range(K):
            nc.sync.dma_start(out=w1bd_sb[:, t, :], in_=w1bd_d[t])
        nc.sync.dma_start(out=w2s_sb[:], in_=w2s_d[:])
        nc.sync.dma_start(out=b1c_sb[:], in_=b1c_d[:])
        nc.sync.dma_start(out=b2c_sb[:], in_=b2c_d[:])
        nc.sync.dma_start(out=wlbl_sb[:], in_=wlbl_d[:])
        nc.sync.dma_start(out=iden_sb[:], in_=iden_d[:])
        for kk in range(4):
            nc.sync.dma_start(out=wvs_sb[:, kk, :], in_=wvs_d[kk * 128:(kk + 1) * 128, :])
        nc.vector.memset(ones_col[:], 1.0)
        nc.vector.memset(ones_row[:], 1.0)
        if use_bv:
            bvr_sb = const.tile([1, C], BF)
            nc.sync.dma_start(out=bvr_sb[:], in_=bvr_d[:])
            bvs_sb = const.tile([1, HEADS], BF)
            nc.sync.dma_start(out=bvs_sb[:], in_=bvs_d[:])
        if use_bo:
            bor_sb = const.tile([1, C], BF)
            nc.sync.dma_start(out=bor_sb[:], in_=bor_d[:])

        # ---- stage A ----
        with ExitStack() as ctxa:
            pa = ctxa.enter_context(tc.tile_pool(name="stA", bufs=1))
            pav = ctxa.enter_context(tc.tile_pool(name="stAv", bufs=3))
            psa = ctxa.enter_context(tc.tile_pool(name="psA", bufs=4, space="PSUM"))
            psa2 = ctxa.enter_context(tc.tile_pool(name="psA2", bufs=2, space="PSUM"))
            psav = ctxa.enter_context(tc.tile_pool(name="psAv", bufs=1, space="PSUM"))
            xT_sb = pa.tile([128, 4, BC * P], BF, tag="xT")
            for kk in range(4):
                nc.sync.dma_start(out=xT_sb[:, kk, :], in_=xT_d[kk * 128:(kk + 1) * 128, :])
            nmm = 5 if use_bv else 4
            for wb in range(BC):
                ps = psa.tile([128, C], F32, tag="psv")
                for kk in range(4):
                    nc.tensor.matmul(
                        ps[:],
                        xT_sb[:, kk, wb * P:(wb + 1) * P],
                        wvT_sb[:, kk, :],
                        start=(kk == 0),
                        stop=(kk == nmm - 1),
                    )
                if use_bv:
                    nc.tensor.matmul(ps[:], ones_row[:], bvr_sb[:], start=False, stop=True)
                ps2 = psa2.tile([128, HEADS], F32, tag="ps2")
                for kk in range(4):
                    nc.tensor.matmul(
                        ps2[:],
                        xT_sb[:, kk, wb * P:(wb + 1) * P],
                        wvs_sb[:, kk, :],
                        start=(kk == 0),
                        stop=(kk == nmm - 1),
                    )
                if use_bv:
                    nc.tensor.matmul(ps2[:], ones_row[:], bvs_sb[:],
                                     start=False, stop=True)
                vtmp = pav.tile([128, C], BF, tag="vtmp")
                nc.scalar.activation(vtmp[:], ps[:], Copy)
                nc.scalar.activation(vpart[:, wb, :], ps2[:], Copy)
                nc.gpsimd.dma_start(out=vdram[wb], in_=vtmp[:])
            pvm = psav.tile([1, BC * HEADS], F32)
            nc.tensor.matmul(
                pvm[:],
                ones_col[:],
                vpart[:].rearrange("p a b -> p (a b)"),
                start=True,
                stop=True,
            )
            for m in range(K):
                nc.vector.tensor_scalar(
                    vrows[:, m, :],
                    pvm[:],
                    wlbl_sb[:, m:m + 1],
                    wlbl_sb[:, K + m:K + m + 1],
                    op0=mult,
                    op1=add,
                )
            for g in range(NG):
                for m in range(K):
                    nc.gpsimd.dma_start(
                        out=vcols[:, g, m:m + 1],
                        in_=vrows[:, m, g * 128:(g + 1) * 128],
                    )

        # ---- stages B + C (interleaved by pair) ----
        pb = ctx.enter_context(tc.tile_pool(name="stB", bufs=1))
        pat = ctx.enter_context(tc.tile_pool(name="stBat", bufs=3))
        pb2 = ctx.enter_context(tc.tile_pool(name="stB2", bufs=1))
        pbp = ctx.enter_context(tc.tile_pool(name="stBp", bufs=1))
        pan = ctx.enter_context(tc.tile_pool(name="stBan", bufs=2))
        psb = ctx.enter_context(tc.tile_pool(name="psB", bufs=3, space="PSUM"))
        pc = ctx.enter_context(tc.tile_pool(name="stC", bufs=8))
        pcr = ctx.enter_context(tc.tile_pool(name="stCr", bufs=4))
        pcv = ctx.enter_context(tc.tile_pool(name="stCv", bufs=3))
        pco = ctx.enter_context(tc.tile_pool(name="stCo", bufs=3))
        psct = ctx.enter_context(tc.tile_pool(name="psCt", bufs=2, space="PSUM"))
        psc = ctx.enter_context(tc.tile_pool(name="psC", bufs=2, space="PSUM"))
        pscf = ctx.enter_context(tc.tile_pool(name="psCf", bufs=1, space="PSUM"))
        nmmo = 5 if use_bo else 4
        a1_ch = [pb.tile([112, ROWS, JP], BF, tag=f"a1ch{i}", name=f"a1ch{i}")
                 for i in range(2)]
        for i in range(2):
            nc.vector.memset(a1_ch[i][:], 0.0)

        def stage_c(B_range):
            for B in B_range:
                pidx = B // (BC // 2)
                B_loc = B % (BC // 2)
                anew_h = anew_p[pidx].tensor
                vB = pcv.tile([128, C], BF, tag="vB", name=f"vB{B}")
                nc.sync.dma_start(out=vB[:], in_=vdram[B])
                anr = []
                for h in range(2):
                    a = pcr.tile([128, 4, JP], BF, tag="anr", name=f"anr{B}_{h}")
                    nc.sync.dma_start(
                        out=a[:],
                        in_=bass.AP(anew_h, (B_loc * HEADS + 4 * h) * P * JP,
                                    [[JP, 128], [P * JP, 4], [1, JP]]),
                    )
                    anr.append(a)
                psoa = psc.tile([128, 4, P], F32, tag="psoa", name=f"psoa{B}")
                for np_ in range(4):
                    pst = psct.tile([128, 2, P], BF, tag="pst", name=f"pst{B}_{np_}")
                    for q in range(2):
                        n = 2 * np_ + q
                        nc.tensor.transpose(pst[:, q, :], anr[n // 4][:, n % 4, 0:P],
                                            iden_sb[:], )
                    anT = pc.tile([128, 2, P], BF, tag="anT", name=f"anT{B}_{np_}")
                    nc.scalar.activation(anT[:], pst[:], Copy)
                    for q in range(2):
                        n = 2 * np_ + q
                        nc.tensor.matmul(
                            psoa[(n % 2) * D:(n % 2 + 1) * D, n // 2, :],
                            vB[:, n * D:(n + 1) * D],
                            anT[:, q, :],
                            start=True,
                            stop=True,
                            skip_group_check=True,
                        )
                of = pc.tile([128, 4, P], BF, tag="of", name=f"of{B}")
                nc.scalar.activation(of[:], psoa[:], Copy)
                psf = pscf.tile([128, C], F32, tag="psf", name=f"psf{B}")
                for kk in range(4):
                    nc.tensor.matmul(
                        psf[:], of[:, kk, :], woT_sb[:, kk, :],
                        start=(kk == 0), stop=(kk == nmmo - 1),
                    )
                if use_bo:
                    nc.tensor.matmul(psf[:], ones_row[:], bor_sb[:],
                                     start=False, stop=True)
                ot = pco.tile([128, C], BF, tag="ot", name=f"ot{B}")
                nc.scalar.activation(ot[:], psf[:], Copy)
                nc.scalar.dma_start(out=outT_t[B], in_=ot[:])

        if True:
            for pr in range(NPAIR):
                for cix in range(NCH):
                    i0 = IC * cix
                    a1 = a1_ch[(pr * NCH + cix) % 2]
                    r_lo = max(i0 - 1, 0)
                    r_hi = min(i0 + ROWS - 1, P)
                    nrows = r_hi - r_lo
                    loc0 = r_lo - (i0 - 1)
                    ats = []
                    for h in range(2):
                        g = 2 * pr + h
                        at = pat.tile([128, ROWS, JP], BF, tag="at",
                                      name=f"at{g}_{cix}")
                        nc.sync.dma_start(
                            out=at[:, loc0:loc0 + nrows, :],
                            in_=attn_d[g * 128:(g + 1) * 128, r_lo * JP:r_hi * JP],
                        )
                        ats.append(at)
                    # conv1: both halves into one psum chunk (partitions 0/64)
                    if i0 == 0:
                        a1_lo, a1_hi = 1, 35
                    elif i0 == P - IC:
                        a1_lo, a1_hi = 0, 33
                    else:
                        a1_lo, a1_hi = 0, 34
                    r = a1_lo
                    while r < a1_hi:
                        nr = min(4, a1_hi - r)
                        ps = psb.tile([128, 4, P], F32, tag="psc1")
                        for h in range(2):
                            for t in range(K):
                                nc.tensor.matmul(
                                    ps[h * 64:h * 64 + GB * K, :nr, :],
                                    w1bd_sb[:, t, :],
                                    ats[h][:, r:r + nr, t:t + P],
                                    start=(t == 0),
                                    stop=(t == K - 1),
                                    skip_group_check=True,
                                )
                        nc.scalar.activation(
                            a1[:, r:r + nr, 1:1 + P],
                            ps[0:112, :nr, :],
                            Relu,
                            bias=b1c_sb[0:112, :],
                        )
                        r += nr
                    if i0 == 0:
                        nc.vector.memset(a1[:, 0, :], 0.0)
                    if i0 == P - IC:
                        nc.vector.memset(a1[:, 33, :], 0.0)
                    a1f = a1[:].rearrange("p a b -> p (a b)")
                    # conv2: 9 flat taps, ts (4x/2x) + in-place tt add (2x)
                    acc = pb2.tile([112, FL], BF, tag="acc", bufs=2,
                                   name=f"acc{pr}_{cix}")
                    nc.vector.tensor_scalar(
                        acc[:], a1f[:, 0:FL], w2s_sb[0:112, 0:1], None, op0=mult)
                    for tap in range(1, 9):
                        di, dj = tap // 3, tap % 3
                        src_ap = a1f[:, di * JP + dj:di * JP + dj + FL]
                        tsc = pb2.tile([112, FL], BF, tag="tsc", bufs=2,
                                       name=f"tsc{pr}_{cix}_{tap}")
                        nc.vector.tensor_scalar(
                            tsc[:], src_ap, w2s_sb[0:112, tap:tap + 1], None, op0=mult)
                        nc.vector.tensor_add(acc[:], tsc[:], acc[:])
                    A_ch = pb2.tile([112, FL], BF, tag="Ach", bufs=2,
                                    name=f"Ach{pr}_{cix}")
                    nc.scalar.activation(A_ch[:], acc[:], Relu, bias=b2c_sb[0:112, :])
                    for h in range(2):
                        g = 2 * pr + h
                        atf = ats[h][:].rearrange("p a b -> p (a b)")
                        arep = pb2.tile([128, K, FL], BF, tag="arep", bufs=1,
                                        name=f"arep{pr}_{cix}_{h}")
                        for m in range(K):
                            for n in range(HEADS):
                                nc.gpsimd.dma_start(
                                    out=arep[n:128:HEADS, m, :],
                                    in_=A_ch[h * 64 + m:h * 64 + GB * K:K, :],
                                )
                        pt0 = pbp.tile([128, FL], BF, tag="pt0", bufs=2,
                                       name=f"pt0{pr}_{cix}_{h}")
                        pt1 = pbp.tile([128, FL], BF, tag="pt1", bufs=2,
                                       name=f"pt1{pr}_{cix}_{h}")
                        nc.vector.tensor_scalar(
                            pt0[:], atf[:, JP:JP + FL], vcols[:, g, 0:1], None, op0=mult)
                        nc.vector.tensor_mul(pt0[:], pt0[:], arep[:, 0, :])
                        nc.vector.tensor_scalar(
                            pt1[:], atf[:, JP + 1:JP + 1 + FL], vcols[:, g, 1:2],
                            None, op0=mult)
                        nc.vector.tensor_mul(pt1[:], pt1[:], arep[:, 1, :])
                        nc.vector.tensor_add(pt0[:], pt0[:], pt1[:])
                        nc.vector.tensor_scalar(
                            pt1[:], atf[:, JP + 2:JP + 2 + FL], vcols[:, g, 2:3],
                            None, op0=mult)
                        nc.vector.tensor_mul(pt1[:], pt1[:], arep[:, 2, :])
                        anew = pan.tile([128, FL], BF, tag="anew",
                                        name=f"anew{pr}_{cix}_{h}")
                        nc.vector.tensor_add(anew[:], pt0[:], pt1[:])
                        nc.scalar.dma_start(
                            out=anew_p[pr][h * 128:(h + 1) * 128,
                                           i0 * JP:i0 * JP + FL],
                            in_=anew[:],
                        )

                    if pr == 1:
                        stage_c(range(cix * 8, cix * 8 + 8))

            stage_c(range(BC // 2, BC))

    nc.compile()
    return nc


LAST_EXEC_NS = None
_NC_CACHE = {}


def _get_nc(use_bv, use_bo):
    key = (use_bv, use_bo)
    if key not in _NC_CACHE:
        _NC_CACHE[key] = build_nc(use_bv, use_bo)
    return _NC_CACHE[key]


def prepare(output, attn, Wv, bv, W1, b1, W2, b2, Wl, bl, Wo, bo, **_):
    output = np.asarray(output, np.float32)
    attn = np.asarray(attn, np.float32)
    Wv = np.asarray(Wv, np.float32)
    bv = np.asarray(bv, np.float32)
    W1 = np.asarray(W1, np.float32)
    b1 = np.asarray(b1, np.float32)
    W2 = np.asarray(W2, np.float32)
    b2 = np.asarray(b2, np.float32)
    Wl = np.asarray(Wl, np.float32)
    bl = np.asarray(bl, np.float32)
    Wo = np.asarray(Wo, np.float32)
    bo = np.asarray(bo, np.float32)

    use_bv = bool(np.any(bv))
    use_bo = bool(np.any(bo))
    nc = _get_nc(use_bv, use_bo)

    w1bd = np.zeros((K, 128, GB * K), np.float32)
    for t in range(K):
        for bi in range(GB):
            for n in range(HEADS):
                for kk in range(K):
                    w1bd[t, bi * HEADS + n, bi * K + kk] = W1[kk, n, 0, t]
    w2s = np.zeros((128, 9), np.float32)
    b1c = np.zeros((128, 1), np.float32)
    b2c = np.zeros((128, 1), np.float32)
    for h in range(2):
        for bi in range(GB):
            for kk in range(K):
                row = h * 64 + bi * K + kk
                w2s[row, :] = W2[kk, 0].reshape(9)
                b1c[row, 0] = b1[kk]
                b2c[row, 0] = b2[kk]
    wlbl = np.concatenate([Wl[:, 0] / (P * D), bl]).reshape(1, 2 * K).astype(np.float32)

    shared = {
        "wvT": np.ascontiguousarray(Wv.T).astype(NPBF),
        "woT": np.ascontiguousarray(Wo.T).astype(NPBF),
        "w1bd": w1bd.astype(NPBF),
        "w2s": w2s,
        "b1c": b1c,
        "b2c": b2c,
        "wlbl": wlbl,
        "iden": np.eye(128, dtype=np.float32).astype(NPBF),
        "wvs": Wv.T.reshape(C, HEADS, D).sum(2).astype(NPBF),
    }
    if use_bv:
        shared["bvr"] = bv.reshape(1, C).astype(NPBF)
        shared["bvs"] = bv.reshape(HEADS, D).sum(1).reshape(1, HEADS).astype(NPBF)
    if use_bo:
        shared["bor"] = bo.reshape(1, C).astype(NPBF)

    attn_bf = attn.astype(NPBF)
    output_bf = output.astype(NPBF)
    in_maps = []
    for k in range(NCORES):
        b_k = k // 2
        ws = (k % 2) * BC
        ap = np.zeros((BC * HEADS, P, JP), NPBF)
        ap[:, :, 1:1 + P] = attn_bf[k * BC:(k + 1) * BC].reshape(BC * HEADS, P, P)
        x_sl = output_bf[b_k, :, :, ws:ws + BC].transpose(0, 2, 1).reshape(C, BC * P)
        m = dict(shared)
        m["attn_in"] = ap.reshape(BC * HEADS, P * JP)
        m["xT"] = np.ascontiguousarray(x_sl)
        in_maps.append(m)
    return nc, in_maps


def kernel(**inputs):
    nc, in_maps = prepare(**inputs)
    import os
    trace = bool(os.environ.get("BASS_TRACE"))
    if trace:
        _install_ntff_hook()
    res = run_bass_kernel_spmd(
        nc, in_maps, core_ids=list(range(NCORES)),
        trace=trace, tmpdir=os.environ.get("BASS_TRACE_DIR") or None,
    )
    global LAST_EXEC_NS
    LAST_EXEC_NS = res.exec_time_ns
    if res.exec_time_ns is not None:
        print(f"HW exec time: {res.exec_time_ns} ns")

    attn_new = np.empty((512, HEADS, P, P), np.float32)
    out = np.empty((4, C, P, P), np.float32)
    for k in range(NCORES):
        r = res.results[k]
        an = np.concatenate(
            [r["attn_new0"], r["attn_new1"]], axis=0
        ).astype(np.float32).reshape(BC, HEADS, P, JP)
        attn_new[k * BC:(k + 1) * BC] = an[:, :, :, 0:P]
        b_k = k // 2
        ws = (k % 2) * BC
        out[b_k, :, :, ws:ws + BC] = r["outT"].astype(np.float32).transpose(2, 1, 0)
    return out, attn_new


def _install_ntff_hook():
    import types
    try:
        import antenv.axon_hooks  # noqa: F401
        return
    except ImportError:
        pass
    try:
        from trn_agent_boot.trn_boot import _ntff_profile_via_ctypes
    except ImportError:
        return
    import antenv
    mod = types.ModuleType("antenv.axon_hooks")
    state = {"hook": None}
    mod.set_axon_ntff_profile_hook = lambda h: state.__setitem__("hook", h)
    mod.get_axon_ntff_profile_hook = lambda: state["hook"]
    sys.modules["antenv.axon_hooks"] = mod
    antenv.axon_hooks = mod
    hook = _ntff_profile_via_ctypes("/opt/axon/libaxon_pjrt.so")
    if hook is not None:
        mod.set_axon_ntff_profile_hook(hook)


# revision 28
# speedup vs baseline: 1.1670x; 1.1670x over previous
"""Adaptive-attention Bass kernel for 8 TRN2 NeuronCores (v2).

Data-parallel over the B = b*w = 512 pseudo-batch: core k handles
B in [64k, 64k+64)  (image b = k//2, w-slice of 64 columns).

Per-core pipeline (bf16 compute, f32 PSUM accumulation):
  A) value conv (PE GEMM) -> v tiles [j | (n,d)] spilled to DRAM; vm/V
  B) per group-pair (2 x 16 B packed on partitions 0-47 / 64-111):
     conv1 (PE, 3 shifted matmuls, block-diag weights) -> relu (ACT)
     -> depthwise conv2 (DVE, flat aligned bf16 scalar_tensor_tensor)
     -> relu (ACT) -> A replicated over heads (gpsimd SWDGE)
     -> involution (DVE stt) -> attn_new (padded rows, ACT DMA ring)
  C) attn_new read back natural, PE-transpose via identity, attn@v (PE),
     out conv (PE GEMM) -> out

Host pads attn rows to 130 cols (zero guards) so loads/stores are fully
contiguous; attn_new returns padded and is stripped on host.
"""

import sys

sys.path.insert(0, "/opt/trn_rl_repo")

from contextlib import ExitStack

import ml_dtypes
import numpy as np

import concourse.bacc as bacc
import concourse.bass as bass
import concourse.tile as tile
from concourse import mybir
from concourse.bass_utils import run_bass_kernel_spmd

BF = mybir.dt.bfloat16
F32 = mybir.dt.float32
NPBF = ml_dtypes.bfloat16

NCORES = 8
BC = 64
HEADS = 8
P = 128
D = 64
C = 512
K = 3
NG = 4
GB = 16
NPAIR = 2
NCH = 4
IC = 32
JP = 130
ROWS = 36
FL = IC * JP  # 4160

mult = mybir.AluOpType.mult
add = mybir.AluOpType.add
Relu = mybir.ActivationFunctionType.Relu
Copy = mybir.ActivationFunctionType.Copy


def build_nc(use_bv: bool, use_bo: bool) -> bass.Bass:
    nc = bacc.Bacc(target_bir_lowering=False)

    attn_d = nc.dram_tensor("attn_in", [BC * HEADS, P * JP], BF, kind="ExternalInput")
    xT_d = nc.dram_tensor("xT", [C, BC * P], BF, kind="ExternalInput")
    wvT_d = nc.dram_tensor("wvT", [C, C], BF, kind="ExternalInput")
    woT_d = nc.dram_tensor("woT", [C, C], BF, kind="ExternalInput")
    w1bd_d = nc.dram_tensor("w1bd", [K, 128, GB * K], BF, kind="ExternalInput")
    w2s_d = nc.dram_tensor("w2s", [128, 9], F32, kind="ExternalInput")
    b1c_d = nc.dram_tensor("b1c", [128, 1], F32, kind="ExternalInput")
    b2c_d = nc.dram_tensor("b2c", [128, 1], F32, kind="ExternalInput")
    wlbl_d = nc.dram_tensor("wlbl", [1, 2 * K], F32, kind="ExternalInput")
    iden_d = nc.dram_tensor("iden", [128, 128], BF, kind="ExternalInput")
    if use_bv:
        bvr_d = nc.dram_tensor("bvr", [1, C], BF, kind="ExternalInput")
    if use_bo:
        bor_d = nc.dram_tensor("bor", [1, C], BF, kind="ExternalInput")

    with tile.TileContext(nc) as tc, ExitStack() as ctx:
        pdram = ctx.enter_context(tc.tile_pool(name="dram", bufs=1, space="DRAM"))
        anew_p = [pdram.tile([BC * HEADS // 2, P * JP], BF, kind="ExternalOutput",
                             name=f"attn_new{i}", uniquify=False)
                  for i in range(2)]
        outT_t = pdram.tile([BC, P, C], BF, kind="ExternalOutput",
                            name="outT", uniquify=False)
        vdram = pdram.tile([BC, 128, C], BF, name="vdram")

        const = ctx.enter_context(tc.tile_pool(name="const", bufs=1))
        wvT_sb = const.tile([128, 4, C], BF)
        woT_sb = const.tile([128, 4, C], BF)
        w1bd_sb = const.tile([128, K, GB * K], BF)
        w2s_sb = const.tile([128, 9], F32)
        b1c_sb = const.tile([128, 1], F32)
        b2c_sb = const.tile([128, 1], F32)
        wlbl_sb = const.tile([1, 2 * K], F32)
        iden_sb = const.tile([128, 128], BF)
        ones_col = const.tile([128, 1], F32)
        ones_row = const.tile([1, 128], BF)
        vrows = const.tile([1, K, C], F32)
        vcols = const.tile([128, NG, K], F32)
        vpart = const.tile([128, BC, HEADS], F32)

        for kk in range(4):
            nc.sync.dma_start(out=wvT_sb[:, kk, :], in_=wvT_d[kk * 128:(kk + 1) * 128, :])
            nc.sync.dma_start(out=woT_sb[:, kk, :], in_=woT_d[kk * 128:(kk + 1) * 128, :])
        for t in range(K):
            nc.sync.dma_start(out=w1bd_sb[:, t, :], in_=w1bd_d[t])
        nc.sync.dma_start(out=w2s_sb[:], in_=w2s_d[:])
        nc.sync.dma_start(out=b1c_sb[:], in_=b1c_d[:])
        nc.sync.dma_start(out=b2c_sb[:], in_=b2c_d[:])
        nc.sync.dma_start(out=wlbl_sb[:], in_=wlbl_d[:])
        nc.sync.dma_start(out=iden_sb[:], in_=iden_d[:])
        nc.vector.memset(ones_col[:], 1.0)
        nc.vector.memset(ones_row[:], 1.0)
        if use_bv:
            bvr_sb = const.tile([1, C], BF)
            nc.sync.dma_start(out=bvr_sb[:], in_=bvr_d[:])
        if use_bo:
            bor_sb = const.tile([1, C], BF)
            nc.sync.dma_start(out=bor_sb[:], in_=bor_d[:])

        # ---- stage A ----
        with ExitStack() as ctxa:
            pa = ctxa.enter_context(tc.tile_pool(name="stA", bufs=1))
            pav = ctxa.enter_context(tc.tile_pool(name="stAv", bufs=3))
            psa = ctxa.enter_context(tc.tile_pool(name="psA", bufs=4, space="PSUM"))
            psav = ctxa.enter_context(tc.tile_pool(name="psAv", bufs=1, space="PSUM"))
            xT_sb = pa.tile([128, 4, BC * P], BF, tag="xT")
            for kk in range(4):
                nc.sync.dma_start(out=xT_sb[:, kk, :], in_=xT_d[kk * 128:(kk + 1) * 128, :])
            nmm = 5 if use_bv else 4
            for wb in range(BC):
                ps = psa.tile([128, C], F32, tag="psv")
                for kk in range(4):
                    nc.tensor.matmul(
                        ps[:],
                        xT_sb[:, kk, wb * P:(wb + 1) * P],
                        wvT_sb[:, kk, :],
                        start=(kk == 0),
                        stop=(kk == nmm - 1),
                    )
                if use_bv:
                    nc.tensor.matmul(ps[:], ones_row[:], bvr_sb[:], start=False, stop=True)
                vtmp = pav.tile([128, C], BF, tag="vtmp")
                nc.scalar.activation(vtmp[:], ps[:], Copy)
                nc.vector.tensor_reduce(
                    vpart[:, wb, :],
                    vtmp[:].rearrange("p (n d) -> p n d", d=D),
                    axis=mybir.AxisListType.X,
                    op=add,
                )
                nc.gpsimd.dma_start(out=vdram[wb], in_=vtmp[:])
            pvm = psav.tile([1, BC * HEADS], F32)
            nc.tensor.matmul(
                pvm[:],
                ones_col[:],
                vpart[:].rearrange("p a b -> p (a b)"),
                start=True,
                stop=True,
            )
            for m in range(K):
                nc.vector.tensor_scalar(
                    vrows[:, m, :],
                    pvm[:],
                    wlbl_sb[:, m:m + 1],
                    wlbl_sb[:, K + m:K + m + 1],
                    op0=mult,
                    op1=add,
                )
            for g in range(NG):
                for m in range(K):
                    nc.gpsimd.dma_start(
                        out=vcols[:, g, m:m + 1],
                        in_=vrows[:, m, g * 128:(g + 1) * 128],
                    )

        # ---- stages B + C (interleaved by pair) ----
        pb = ctx.enter_context(tc.tile_pool(name="stB", bufs=1))
        pat = ctx.enter_context(tc.tile_pool(name="stBat", bufs=3))
        pb2 = ctx.enter_context(tc.tile_pool(name="stB2", bufs=1))
        pbp = ctx.enter_context(tc.tile_pool(name="stBp", bufs=1))
        pan = ctx.enter_context(tc.tile_pool(name="stBan", bufs=2))
        psb = ctx.enter_context(tc.tile_pool(name="psB", bufs=3, space="PSUM"))
        pc = ctx.enter_context(tc.tile_pool(name="stC", bufs=8))
        pcr = ctx.enter_context(tc.tile_pool(name="stCr", bufs=4))
        pcv = ctx.enter_context(tc.tile_pool(name="stCv", bufs=3))
        pco = ctx.enter_context(tc.tile_pool(name="stCo", bufs=3))
        psct = ctx.enter_context(tc.tile_pool(name="psCt", bufs=2, space="PSUM"))
        psc = ctx.enter_context(tc.tile_pool(name="psC", bufs=2, space="PSUM"))
        pscf = ctx.enter_context(tc.tile_pool(name="psCf", bufs=1, space="PSUM"))
        nmmo = 5 if use_bo else 4
        a1_ch = [pb.tile([112, ROWS, JP], BF, tag=f"a1ch{i}", name=f"a1ch{i}")
                 for i in range(2)]
        for i in range(2):
            nc.vector.memset(a1_ch[i][:], 0.0)

        def stage_c(B_range):
            for B in B_range:
                pidx = B // (BC // 2)
                B_loc = B % (BC // 2)
                anew_h = anew_p[pidx].tensor
                vB = pcv.tile([128, C], BF, tag="vB", name=f"vB{B}")
                nc.sync.dma_start(out=vB[:], in_=vdram[B])
                anr = []
                for h in range(2):
                    a = pcr.tile([128, 4, JP], BF, tag="anr", name=f"anr{B}_{h}")
                    nc.sync.dma_start(
                        out=a[:],
                        in_=bass.AP(anew_h, (B_loc * HEADS + 4 * h) * P * JP,
                                    [[JP, 128], [P * JP, 4], [1, JP]]),
                    )
                    anr.append(a)
                psoa = psc.tile([128, 4, P], F32, tag="psoa", name=f"psoa{B}")
                for np_ in range(4):
                    pst = psct.tile([128, 2, P], BF, tag="pst", name=f"pst{B}_{np_}")
                    for q in range(2):
                        n = 2 * np_ + q
                        nc.tensor.transpose(pst[:, q, :], anr[n // 4][:, n % 4, 0:P],
                                            iden_sb[:], )
                    anT = pc.tile([128, 2, P], BF, tag="anT", name=f"anT{B}_{np_}")
                    nc.scalar.activation(anT[:], pst[:], Copy)
                    for q in range(2):
                        n = 2 * np_ + q
                        nc.tensor.matmul(
                            psoa[(n % 2) * D:(n % 2 + 1) * D, n // 2, :],
                            vB[:, n * D:(n + 1) * D],
                            anT[:, q, :],
                            start=True,
                            stop=True,
                            skip_group_check=True,
                        )
                of = pc.tile([128, 4, P], BF, tag="of", name=f"of{B}")
                nc.scalar.activation(of[:], psoa[:], Copy)
                psf = pscf.tile([128, C], F32, tag="psf", name=f"psf{B}")
                for kk in range(4):
                    nc.tensor.matmul(
                        psf[:], of[:, kk, :], woT_sb[:, kk, :],
                        start=(kk == 0), stop=(kk == nmmo - 1),
                    )
                if use_bo:
                    nc.tensor.matmul(psf[:], ones_row[:], bor_sb[:],
                                     start=False, stop=True)
                ot = pco.tile([128, C], BF, tag="ot", name=f"ot{B}")
                nc.scalar.activation(ot[:], psf[:], Copy)
                nc.scalar.dma_start(out=outT_t[B], in_=ot[:])

        if True:
            for pr in range(NPAIR):
                for cix in range(NCH):
                    i0 = IC * cix
                    a1 = a1_ch[(pr * NCH + cix) % 2]
                    r_lo = max(i0 - 1, 0)
                    r_hi = min(i0 + ROWS - 1, P)
                    nrows = r_hi - r_lo
                    loc0 = r_lo - (i0 - 1)
                    ats = []
                    for h in range(2):
                        g = 2 * pr + h
                        at = pat.tile([128, ROWS, JP], BF, tag="at",
                                      name=f"at{g}_{cix}")
                        nc.sync.dma_start(
                            out=at[:, loc0:loc0 + nrows, :],
                            in_=attn_d[g * 128:(g + 1) * 128, r_lo * JP:r_hi * JP],
                        )
                        ats.append(at)
                    # conv1: both halves into one psum chunk (partitions 0/64)
                    if i0 == 0:
                        a1_lo, a1_hi = 1, 35
                    elif i0 == P - IC:
                        a1_lo, a1_hi = 0, 33
                    else:
                        a1_lo, a1_hi = 0, 34
                    r = a1_lo
                    while r < a1_hi:
                        nr = min(4, a1_hi - r)
                        ps = psb.tile([128, 4, P], F32, tag="psc1")
                        for h in range(2):
                            for t in range(K):
                                nc.tensor.matmul(
                                    ps[h * 64:h * 64 + GB * K, :nr, :],
                                    w1bd_sb[:, t, :],
                                    ats[h][:, r:r + nr, t:t + P],
                                    start=(t == 0),
                                    stop=(t == K - 1),
                                    skip_group_check=True,
                                )
                        nc.scalar.activation(
                            a1[:, r:r + nr, 1:1 + P],
                            ps[0:112, :nr, :],
                            Relu,
                            bias=b1c_sb[0:112, :],
                        )
                        r += nr
                    if i0 == 0:
                        nc.vector.memset(a1[:, 0, :], 0.0)
                    if i0 == P - IC:
                        nc.vector.memset(a1[:, 33, :], 0.0)
                    a1f = a1[:].rearrange("p a b -> p (a b)")
                    # conv2: 9 flat taps, ts (4x/2x) + in-place tt add (2x)
                    acc = pb2.tile([112, FL], BF, tag="acc", bufs=2,
                                   name=f"acc{pr}_{cix}")
                    nc.vector.tensor_scalar(
                        acc[:], a1f[:, 0:FL], w2s_sb[0:112, 0:1], None, op0=mult)
                    for tap in range(1, 9):
                        di, dj = tap // 3, tap % 3
                        src_ap = a1f[:, di * JP + dj:di * JP + dj + FL]
                        tsc = pb2.tile([112, FL], BF, tag="tsc", bufs=2,
                                       name=f"tsc{pr}_{cix}_{tap}")
                        nc.vector.tensor_scalar(
                            tsc[:], src_ap, w2s_sb[0:112, tap:tap + 1], None, op0=mult)
                        nc.vector.tensor_add(acc[:], tsc[:], acc[:])
                    A_ch = pb2.tile([112, FL], BF, tag="Ach", bufs=2,
                                    name=f"Ach{pr}_{cix}")
                    nc.scalar.activation(A_ch[:], acc[:], Relu, bias=b2c_sb[0:112, :])
                    for h in range(2):
                        g = 2 * pr + h
                        atf = ats[h][:].rearrange("p a b -> p (a b)")
                        arep = pb2.tile([128, K, FL], BF, tag="arep", bufs=1,
                                        name=f"arep{pr}_{cix}_{h}")
                        for m in range(K):
                            for n in range(HEADS):
                                nc.gpsimd.dma_start(
                                    out=arep[n:128:HEADS, m, :],
                                    in_=A_ch[h * 64 + m:h * 64 + GB * K:K, :],
                                )
                        pt0 = pbp.tile([128, FL], BF, tag="pt0", bufs=2,
                                       name=f"pt0{pr}_{cix}_{h}")
                        pt1 = pbp.tile([128, FL], BF, tag="pt1", bufs=2,
                                       name=f"pt1{pr}_{cix}_{h}")
                        nc.vector.tensor_scalar(
                            pt0[:], atf[:, JP:JP + FL], vcols[:, g, 0:1], None, op0=mult)
                        nc.vector.tensor_mul(pt0[:], pt0[:], arep[:, 0, :])
                        nc.vector.tensor_scalar(
                            pt1[:], atf[:, JP + 1:JP + 1 + FL], vcols[:, g, 1:2],
                            None, op0=mult)
                        nc.vector.tensor_mul(pt1[:], pt1[:], arep[:, 1, :])
                        nc.vector.tensor_add(pt0[:], pt0[:], pt1[:])
                        nc.vector.tensor_scalar(
                            pt1[:], atf[:, JP + 2:JP + 2 + FL], vcols[:, g, 2:3],
                            None, op0=mult)
                        nc.vector.tensor_mul(pt1[:], pt1[:], arep[:, 2, :])
                        anew = pan.tile([128, FL], BF, tag="anew",
                                        name=f"anew{pr}_{cix}_{h}")
                        nc.vector.tensor_add(anew[:], pt0[:], pt1[:])
                        nc.scalar.dma_start(
                            out=anew_p[pr][h * 128:(h + 1) * 128,
                                           i0 * JP:i0 * JP + FL],
                            in_=anew[:],
                        )

                stage_c(range(pr * (BC // 2), (pr + 1) * (BC // 2)))

    nc.compile()
    return nc


LAST_EXEC_NS = None
_NC_CACHE = {}


def _get_nc(use_bv, use_bo):
    key = (use_bv, use_bo)
    if key not in _NC_CACHE:
        _NC_CACHE[key] = build_nc(use_bv, use_bo)
    return _NC_CACHE[key]


def prepare(output, attn, Wv, bv, W1, b1, W2, b2, Wl, bl, Wo, bo, **_):
    output = np.asarray(output, np.float32)
    attn = np.asarray(attn, np.float32)
    Wv = np.asarray(Wv, np.float32)
    bv = np.asarray(bv, np.float32)
    W1 = np.asarray(W1, np.float32)
    b1 = np.asarray(b1, np.float32)
    W2 = np.asarray(W2, np.float32)
    b2 = np.asarray(b2, np.float32)
    Wl = np.asarray(Wl, np.float32)
    bl = np.asarray(bl, np.float32)
    Wo = np.asarray(Wo, np.float32)
    bo = np.asarray(bo, np.float32)

    use_bv = bool(np.any(bv))
    use_bo = bool(np.any(bo))
    nc = _get_nc(use_bv, use_bo)

    w1bd = np.zeros((K, 128, GB * K), np.float32)
    for t in range(K):
        for bi in range(GB):
            for n in range(HEADS):
                for kk in range(K):
                    w1bd[t, bi * HEADS + n, bi * K + kk] = W1[kk, n, 0, t]
    w2s = np.zeros((128, 9), np.float32)
    b1c = np.zeros((128, 1), np.float32)
    b2c = np.zeros((128, 1), np.float32)
    for h in range(2):
        for bi in range(GB):
            for kk in range(K):
                row = h * 64 + bi * K + kk
                w2s[row, :] = W2[kk, 0].reshape(9)
                b1c[row, 0] = b1[kk]
                b2c[row, 0] = b2[kk]
    wlbl = np.concatenate([Wl[:, 0] / (P * D), bl]).reshape(1, 2 * K).astype(np.float32)

    shared = {
        "wvT": np.ascontiguousarray(Wv.T).astype(NPBF),
        "woT": np.ascontiguousarray(Wo.T).astype(NPBF),
        "w1bd": w1bd.astype(NPBF),
        "w2s": w2s,
        "b1c": b1c,
        "b2c": b2c,
        "wlbl": wlbl,
        "iden": np.eye(128, dtype=np.float32).astype(NPBF),
    }
    if use_bv:
        shared["bvr"] = bv.reshape(1, C).astype(NPBF)
    if use_bo:
        shared["bor"] = bo.reshape(1, C).astype(NPBF)

    attn_bf = attn.astype(NPBF)
    output_bf = output.astype(NPBF)
    in_maps = []
    for k in range(NCORES):
        b_k = k // 2
        ws = (k % 2) * BC
        ap = np.zeros((BC * HEADS, P, JP), NPBF)
        ap[:, :, 1:1 + P] = attn_bf[k * BC:(k + 1) * BC].reshape(BC * HEADS, P, P)
        x_sl = output_bf[b_k, :, :, ws:ws + BC].transpose(0, 2, 1).reshape(C, BC * P)
        m = dict(shared)
        m["attn_in"] = ap.reshape(BC * HEADS, P * JP)
        m["xT"] = np.ascontiguousarray(x_sl)
        in_maps.append(m)
    return nc, in_maps


def kernel(**inputs):
    nc, in_maps = prepare(**inputs)
    import os
    trace = bool(os.environ.get("BASS_TRACE"))
    if trace:
        _install_ntff_hook()
    res = run_bass_kernel_spmd(
        nc, in_maps, core_ids=list(range(NCORES)),
        trace=trace, tmpdir=os.environ.get("BASS_TRACE_DIR") or None,
    )
    global LAST_EXEC_NS
    LAST_EXEC_NS = res.exec_time_ns
    if res.exec_time_ns is not None:
        print(f"HW exec time: {res.exec_time_ns} ns")

    attn_new = np.empty((512, HEADS, P, P), np.float32)
    out = np.empty((4, C, P, P), np.float32)
    for k in range(NCORES):
        r = res.results[k]
        an = np.concatenate(
            [r["attn_new0"], r["attn_new1"]], axis=0
        ).astype(np.float32).reshape(BC, HEADS, P, JP)
        attn_new[k * BC:(k + 1) * BC] = an[:, :, :, 0:P]
        b_k = k // 2
        ws = (k % 2) * BC
        out[b_k, :, :, ws:ws + BC] = r["outT"].astype(np.float32).transpose(2, 1, 0)
    return out, attn_new


def _install_ntff_hook():
    import types
    try:
        import antenv.axon_hooks  # noqa: F401
        return
    except ImportError:
        pass
    try:
        from trn_agent_boot.trn_boot import _ntff_profile_via_ctypes
    except ImportError:
        return
    import antenv
    mod = types.ModuleType("antenv.axon_hooks")
    state = {"hook": None}
    mod.set_axon_ntff_profile_hook = lambda h: state.__setitem__("hook", h)
    mod.get_axon_ntff_profile_hook = lambda: state["hook"]
    sys.modules["antenv.axon_hooks"] = mod
    antenv.axon_hooks = mod
    hook = _ntff_profile_via_ctypes("/opt/axon/libaxon_pjrt.so")
    if hook is not None:
        mod.set_axon_ntff_profile_hook(hook)


# revision 32
# speedup vs baseline: 1.1729x; 1.0050x over previous
"""Adaptive-attention Bass kernel for 8 TRN2 NeuronCores (v2).

Data-parallel over the B = b*w = 512 pseudo-batch: core k handles
B in [64k, 64k+64)  (image b = k//2, w-slice of 64 columns).

Per-core pipeline (bf16 compute, f32 PSUM accumulation):
  A) value conv (PE GEMM) -> v tiles [j | (n,d)] spilled to DRAM; vm/V
  B) per group-pair (2 x 16 B packed on partitions 0-47 / 64-111):
     conv1 (PE, 3 shifted matmuls, block-diag weights) -> relu (ACT)
     -> depthwise conv2 (DVE, flat aligned bf16 scalar_tensor_tensor)
     -> relu (ACT) -> A replicated over heads (gpsimd SWDGE)
     -> involution (DVE stt) -> attn_new (padded rows, ACT DMA ring)
  C) attn_new read back natural, PE-transpose via identity, attn@v (PE),
     out conv (PE GEMM) -> out

Host pads attn rows to 130 cols (zero guards) so loads/stores are fully
contiguous; attn_new returns padded and is stripped on host.
"""

import sys

sys.path.insert(0, "/opt/trn_rl_repo")

from contextlib import ExitStack

import ml_dtypes
import numpy as np

import concourse.bacc as bacc
import concourse.bass as bass
import concourse.tile as tile
from concourse import mybir
from concourse.bass_utils import run_bass_kernel_spmd

BF = mybir.dt.bfloat16
F32 = mybir.dt.float32
NPBF = ml_dtypes.bfloat16

NCORES = 8
BC = 64
HEADS = 8
P = 128
D = 64
C = 512
K = 3
NG = 4
GB = 16
NPAIR = 2
NCH = 4
IC = 32
JP = 130
ROWS = 36
FL = IC * JP  # 4160

mult = mybir.AluOpType.mult
add = mybir.AluOpType.add
Relu = mybir.ActivationFunctionType.Relu
Copy = mybir.ActivationFunctionType.Copy


def build_nc(use_bv: bool, use_bo: bool) -> bass.Bass:
    nc = bacc.Bacc(target_bir_lowering=False)

    attn_d = nc.dram_tensor("attn_in", [BC * HEADS, P * JP], BF, kind="ExternalInput")
    xT_d = nc.dram_tensor("xT", [C, BC * P], BF, kind="ExternalInput")
    wvT_d = nc.dram_tensor("wvT", [C, C], BF, kind="ExternalInput")
    woT_d = nc.dram_tensor("woT", [C, C], BF, kind="ExternalInput")
    w1bd_d = nc.dram_tensor("w1bd", [K, 128, GB * K], BF, kind="ExternalInput")
    w2s_d = nc.dram_tensor("w2s", [128, 9], F32, kind="ExternalInput")
    b1c_d = nc.dram_tensor("b1c", [128, 1], F32, kind="ExternalInput")
    b2c_d = nc.dram_tensor("b2c", [128, 1], F32, kind="ExternalInput")
    wlbl_d = nc.dram_tensor("wlbl", [1, 2 * K], F32, kind="ExternalInput")
    iden_d = nc.dram_tensor("iden", [128, 128], BF, kind="ExternalInput")
    if use_bv:
        bvr_d = nc.dram_tensor("bvr", [1, C], BF, kind="ExternalInput")
    if use_bo:
        bor_d = nc.dram_tensor("bor", [1, C], BF, kind="ExternalInput")

    with tile.TileContext(nc) as tc, ExitStack() as ctx:
        pdram = ctx.enter_context(tc.tile_pool(name="dram", bufs=1, space="DRAM"))
        anew_p = [pdram.tile([BC * HEADS // 2, P * JP], BF, kind="ExternalOutput",
                             name=f"attn_new{i}", uniquify=False)
                  for i in range(2)]
        outT_t = pdram.tile([BC, P, C], BF, kind="ExternalOutput",
                            name="outT", uniquify=False)
        vdram = pdram.tile([BC, 128, C], BF, name="vdram")

        const = ctx.enter_context(tc.tile_pool(name="const", bufs=1))
        wvT_sb = const.tile([128, 4, C], BF)
        woT_sb = const.tile([128, 4, C], BF)
        w1bd_sb = const.tile([128, K, GB * K], BF)
        w2s_sb = const.tile([128, 9], F32)
        b1c_sb = const.tile([128, 1], F32)
        b2c_sb = const.tile([128, 1], F32)
        wlbl_sb = const.tile([1, 2 * K], F32)
        iden_sb = const.tile([128, 128], BF)
        ones_col = const.tile([128, 1], F32)
        ones_row = const.tile([1, 128], BF)
        vrows = const.tile([1, K, C], F32)
        vcols = const.tile([128, NG, K], F32)
        vpart = const.tile([128, BC, HEADS], F32)

        for kk in range(4):
            nc.sync.dma_start(out=wvT_sb[:, kk, :], in_=wvT_d[kk * 128:(kk + 1) * 128, :])
            nc.sync.dma_start(out=woT_sb[:, kk, :], in_=woT_d[kk * 128:(kk + 1) * 128, :])
        for t in range(K):
            nc.sync.dma_start(out=w1bd_sb[:, t, :], in_=w1bd_d[t])
        nc.sync.dma_start(out=w2s_sb[:], in_=w2s_d[:])
        nc.sync.dma_start(out=b1c_sb[:], in_=b1c_d[:])
        nc.sync.dma_start(out=b2c_sb[:], in_=b2c_d[:])
        nc.sync.dma_start(out=wlbl_sb[:], in_=wlbl_d[:])
        nc.sync.dma_start(out=iden_sb[:], in_=iden_d[:])
        nc.vector.memset(ones_col[:], 1.0)
        nc.vector.memset(ones_row[:], 1.0)
        if use_bv:
            bvr_sb = const.tile([1, C], BF)
            nc.sync.dma_start(out=bvr_sb[:], in_=bvr_d[:])
        if use_bo:
            bor_sb = const.tile([1, C], BF)
            nc.sync.dma_start(out=bor_sb[:], in_=bor_d[:])

        # ---- stage A ----
        with ExitStack() as ctxa:
            pa = ctxa.enter_context(tc.tile_pool(name="stA", bufs=1))
            pav = ctxa.enter_context(tc.tile_pool(name="stAv", bufs=3))
            psa = ctxa.enter_context(tc.tile_pool(name="psA", bufs=4, space="PSUM"))
            psav = ctxa.enter_context(tc.tile_pool(name="psAv", bufs=1, space="PSUM"))
            xT_sb = pa.tile([128, 4, BC * P], BF, tag="xT")
            for kk in range(4):
                nc.sync.dma_start(out=xT_sb[:, kk, :], in_=xT_d[kk * 128:(kk + 1) * 128, :])
            nmm = 5 if use_bv else 4
            for wb in range(BC):
                ps = psa.tile([128, C], F32, tag="psv")
                for kk in range(4):
                    nc.tensor.matmul(
                        ps[:],
                        xT_sb[:, kk, wb * P:(wb + 1) * P],
                        wvT_sb[:, kk, :],
                        start=(kk == 0),
                        stop=(kk == nmm - 1),
                    )
                if use_bv:
                    nc.tensor.matmul(ps[:], ones_row[:], bvr_sb[:], start=False, stop=True)
                vtmp = pav.tile([128, C], BF, tag="vtmp")
                nc.scalar.activation(vtmp[:], ps[:], Copy)
                nc.vector.tensor_reduce(
                    vpart[:, wb, :],
                    vtmp[:].rearrange("p (n d) -> p n d", d=D),
                    axis=mybir.AxisListType.X,
                    op=add,
                )
                nc.gpsimd.dma_start(out=vdram[wb], in_=vtmp[:])
            pvm = psav.tile([1, BC * HEADS], F32)
            nc.tensor.matmul(
                pvm[:],
                ones_col[:],
                vpart[:].rearrange("p a b -> p (a b)"),
                start=True,
                stop=True,
            )
            for m in range(K):
                nc.vector.tensor_scalar(
                    vrows[:, m, :],
                    pvm[:],
                    wlbl_sb[:, m:m + 1],
                    wlbl_sb[:, K + m:K + m + 1],
                    op0=mult,
                    op1=add,
                )
            for g in range(NG):
                for m in range(K):
                    nc.gpsimd.dma_start(
                        out=vcols[:, g, m:m + 1],
                        in_=vrows[:, m, g * 128:(g + 1) * 128],
                    )

        # ---- stages B + C (interleaved by pair) ----
        pb = ctx.enter_context(tc.tile_pool(name="stB", bufs=1))
        pat = ctx.enter_context(tc.tile_pool(name="stBat", bufs=3))
        pb2 = ctx.enter_context(tc.tile_pool(name="stB2", bufs=1))
        pbp = ctx.enter_context(tc.tile_pool(name="stBp", bufs=1))
        pan = ctx.enter_context(tc.tile_pool(name="stBan", bufs=2))
        psb = ctx.enter_context(tc.tile_pool(name="psB", bufs=3, space="PSUM"))
        pc = ctx.enter_context(tc.tile_pool(name="stC", bufs=8))
        pcr = ctx.enter_context(tc.tile_pool(name="stCr", bufs=4))
        pcv = ctx.enter_context(tc.tile_pool(name="stCv", bufs=3))
        pco = ctx.enter_context(tc.tile_pool(name="stCo", bufs=3))
        psct = ctx.enter_context(tc.tile_pool(name="psCt", bufs=2, space="PSUM"))
        psc = ctx.enter_context(tc.tile_pool(name="psC", bufs=2, space="PSUM"))
        pscf = ctx.enter_context(tc.tile_pool(name="psCf", bufs=1, space="PSUM"))
        nmmo = 5 if use_bo else 4
        a1_ch = [pb.tile([112, ROWS, JP], BF, tag=f"a1ch{i}", name=f"a1ch{i}")
                 for i in range(2)]
        for i in range(2):
            nc.vector.memset(a1_ch[i][:], 0.0)

        def stage_c(B_range):
            for B in B_range:
                pidx = B // (BC // 2)
                B_loc = B % (BC // 2)
                anew_h = anew_p[pidx].tensor
                vB = pcv.tile([128, C], BF, tag="vB", name=f"vB{B}")
                nc.sync.dma_start(out=vB[:], in_=vdram[B])
                anr = []
                for h in range(2):
                    a = pcr.tile([128, 4, JP], BF, tag="anr", name=f"anr{B}_{h}")
                    nc.sync.dma_start(
                        out=a[:],
                        in_=bass.AP(anew_h, (B_loc * HEADS + 4 * h) * P * JP,
                                    [[JP, 128], [P * JP, 4], [1, JP]]),
                    )
                    anr.append(a)
                psoa = psc.tile([128, 4, P], F32, tag="psoa", name=f"psoa{B}")
                for np_ in range(4):
                    pst = psct.tile([128, 2, P], BF, tag="pst", name=f"pst{B}_{np_}")
                    for q in range(2):
                        n = 2 * np_ + q
                        nc.tensor.transpose(pst[:, q, :], anr[n // 4][:, n % 4, 0:P],
                                            iden_sb[:], )
                    anT = pc.tile([128, 2, P], BF, tag="anT", name=f"anT{B}_{np_}")
                    nc.scalar.activation(anT[:], pst[:], Copy)
                    for q in range(2):
                        n = 2 * np_ + q
                        nc.tensor.matmul(
                            psoa[(n % 2) * D:(n % 2 + 1) * D, n // 2, :],
                            vB[:, n * D:(n + 1) * D],
                            anT[:, q, :],
                            start=True,
                            stop=True,
                            skip_group_check=True,
                        )
                of = pc.tile([128, 4, P], BF, tag="of", name=f"of{B}")
                nc.scalar.activation(of[:], psoa[:], Copy)
                psf = pscf.tile([128, C], F32, tag="psf", name=f"psf{B}")
                for kk in range(4):
                    nc.tensor.matmul(
                        psf[:], of[:, kk, :], woT_sb[:, kk, :],
                        start=(kk == 0), stop=(kk == nmmo - 1),
                    )
                if use_bo:
                    nc.tensor.matmul(psf[:], ones_row[:], bor_sb[:],
                                     start=False, stop=True)
                ot = pco.tile([128, C], BF, tag="ot", name=f"ot{B}")
                nc.scalar.activation(ot[:], psf[:], Copy)
                nc.scalar.dma_start(out=outT_t[B], in_=ot[:])

        if True:
            for pr in range(NPAIR):
                for cix in range(NCH):
                    i0 = IC * cix
                    a1 = a1_ch[(pr * NCH + cix) % 2]
                    r_lo = max(i0 - 1, 0)
                    r_hi = min(i0 + ROWS - 1, P)
                    nrows = r_hi - r_lo
                    loc0 = r_lo - (i0 - 1)
                    ats = []
                    for h in range(2):
                        g = 2 * pr + h
                        at = pat.tile([128, ROWS, JP], BF, tag="at",
                                      name=f"at{g}_{cix}")
                        nc.sync.dma_start(
                            out=at[:, loc0:loc0 + nrows, :],
                            in_=attn_d[g * 128:(g + 1) * 128, r_lo * JP:r_hi * JP],
                        )
                        ats.append(at)
                    # conv1: both halves into one psum chunk (partitions 0/64)
                    if i0 == 0:
                        a1_lo, a1_hi = 1, 35
                    elif i0 == P - IC:
                        a1_lo, a1_hi = 0, 33
                    else:
                        a1_lo, a1_hi = 0, 34
                    r = a1_lo
                    while r < a1_hi:
                        nr = min(4, a1_hi - r)
                        ps = psb.tile([128, 4, P], F32, tag="psc1")
                        for h in range(2):
                            for t in range(K):
                                nc.tensor.matmul(
                                    ps[h * 64:h * 64 + GB * K, :nr, :],
                                    w1bd_sb[:, t, :],
                                    ats[h][:, r:r + nr, t:t + P],
                                    start=(t == 0),
                                    stop=(t == K - 1),
                                    skip_group_check=True,
                                )
                        nc.scalar.activation(
                            a1[:, r:r + nr, 1:1 + P],
                            ps[0:112, :nr, :],
                            Relu,
                            bias=b1c_sb[0:112, :],
                        )
                        r += nr
                    if i0 == 0:
                        nc.vector.memset(a1[:, 0, :], 0.0)
                    if i0 == P - IC:
                        nc.vector.memset(a1[:, 33, :], 0.0)
                    a1f = a1[:].rearrange("p a b -> p (a b)")
                    # conv2: 9 flat taps, ts (4x/2x) + in-place tt add (2x)
                    acc = pb2.tile([112, FL], BF, tag="acc", bufs=2,
                                   name=f"acc{pr}_{cix}")
                    nc.vector.tensor_scalar(
                        acc[:], a1f[:, 0:FL], w2s_sb[0:112, 0:1], None, op0=mult)
                    for tap in range(1, 9):
                        di, dj = tap // 3, tap % 3
                        src_ap = a1f[:, di * JP + dj:di * JP + dj + FL]
                        tsc = pb2.tile([112, FL], BF, tag="tsc", bufs=2,
                                       name=f"tsc{pr}_{cix}_{tap}")
                        nc.vector.tensor_scalar(
                            tsc[:], src_ap, w2s_sb[0:112, tap:tap + 1], None, op0=mult)
                        nc.vector.tensor_add(acc[:], tsc[:], acc[:])
                    A_ch = pb2.tile([112, FL], BF, tag="Ach", bufs=2,
                                    name=f"Ach{pr}_{cix}")
                    nc.scalar.activation(A_ch[:], acc[:], Relu, bias=b2c_sb[0:112, :])
                    for h in range(2):
                        g = 2 * pr + h
                        atf = ats[h][:].rearrange("p a b -> p (a b)")
                        arep = pb2.tile([128, K, FL], BF, tag="arep", bufs=1,
                                        name=f"arep{pr}_{cix}_{h}")
                        for m in range(K):
                            for n in range(HEADS):
                                nc.gpsimd.dma_start(
                                    out=arep[n:128:HEADS, m, :],
                                    in_=A_ch[h * 64 + m:h * 64 + GB * K:K, :],
                                )
                        pt0 = pbp.tile([128, FL], BF, tag="pt0", bufs=2,
                                       name=f"pt0{pr}_{cix}_{h}")
                        pt1 = pbp.tile([128, FL], BF, tag="pt1", bufs=2,
                                       name=f"pt1{pr}_{cix}_{h}")
                        nc.vector.tensor_scalar(
                            pt0[:], atf[:, JP:JP + FL], vcols[:, g, 0:1], None, op0=mult)
                        nc.vector.tensor_mul(pt0[:], pt0[:], arep[:, 0, :])
                        nc.vector.tensor_scalar(
                            pt1[:], atf[:, JP + 1:JP + 1 + FL], vcols[:, g, 1:2],
                            None, op0=mult)
                        nc.vector.tensor_mul(pt1[:], pt1[:], arep[:, 1, :])
                        nc.vector.tensor_add(pt0[:], pt0[:], pt1[:])
                        nc.vector.tensor_scalar(
                            pt1[:], atf[:, JP + 2:JP + 2 + FL], vcols[:, g, 2:3],
                            None, op0=mult)
                        nc.vector.tensor_mul(pt1[:], pt1[:], arep[:, 2, :])
                        anew = pan.tile([128, FL], BF, tag="anew",
                                        name=f"anew{pr}_{cix}_{h}")
                        nc.vector.tensor_add(anew[:], pt0[:], pt1[:])
                        nc.scalar.dma_start(
                            out=anew_p[pr][h * 128:(h + 1) * 128,
                                           i0 * JP:i0 * JP + FL],
                            in_=anew[:],
                        )

                stage_c(range(pr * (BC // 2), (pr + 1) * (BC // 2)))

    nc.compile()
    return nc


LAST_EXEC_NS = None
_NC_CACHE = {}


def _get_nc(use_bv, use_bo):
    key = (use_bv, use_bo)
    if key not in _NC_CACHE:
        _NC_CACHE[key] = build_nc(use_bv, use_bo)
    return _NC_CACHE[key]


def prepare(output, attn, Wv, bv, W1, b1, W2, b2, Wl, bl, Wo, bo, **_):
    output = np.asarray(output, np.float32)
    attn = np.asarray(attn, np.float32)
    Wv = np.asarray(Wv, np.float32)
    bv = np.asarray(bv, np.float32)
    W1 = np.asarray(W1, np.float32)
    b1 = np.asarray(b1, np.float32)
    W2 = np.asarray(W2, np.float32)
    b2 = np.asarray(b2, np.float32)
    Wl = np.asarray(Wl, np.float32)
    bl = np.asarray(bl, np.float32)
    Wo = np.asarray(Wo, np.float32)
    bo = np.asarray(bo, np.float32)

    use_bv = bool(np.any(bv))
    use_bo = bool(np.any(bo))
    nc = _get_nc(use_bv, use_bo)

    w1bd = np.zeros((K, 128, GB * K), np.float32)
    for t in range(K):
        for bi in range(GB):
            for n in range(HEADS):
                for kk in range(K):
                    w1bd[t, bi * HEADS + n, bi * K + kk] = W1[kk, n, 0, t]
    w2s = np.zeros((128, 9), np.float32)
    b1c = np.zeros((128, 1), np.float32)
    b2c = np.zeros((128, 1), np.float32)
    for h in range(2):
        for bi in range(GB):
            for kk in range(K):
                row = h * 64 + bi * K + kk
                w2s[row, :] = W2[kk, 0].reshape(9)
                b1c[row, 0] = b1[kk]
                b2c[row, 0] = b2[kk]
    wlbl = np.concatenate([Wl[:, 0] / (P * D), bl]).reshape(1, 2 * K).astype(np.float32)

    shared = {
        "wvT": np.ascontiguousarray(Wv.T).astype(NPBF),
        "woT": np.ascontiguousarray(Wo.T).astype(NPBF),
        "w1bd": w1bd.astype(NPBF),
        "w2s": w2s,
        "b1c": b1c,
        "b2c": b2c,
        "wlbl": wlbl,
        "iden": np.eye(128, dtype=np.float32).astype(NPBF),
    }
    if use_bv:
        shared["bvr"] = bv.reshape(1, C).astype(NPBF)
    if use_bo:
        shared["bor"] = bo.reshape(1, C).astype(NPBF)

    attn_bf = attn.astype(NPBF)
    output_bf = output.astype(NPBF)
    in_maps = []
    for k in range(NCORES):
        b_k = k // 2
        ws = (k % 2) * BC
        ap = np.zeros((BC * HEADS, P, JP), NPBF)
        ap[:, :, 1:1 + P] = attn_bf[k * BC:(k + 1) * BC].reshape(BC * HEADS, P, P)
        x_sl = output_bf[b_k, :, :, ws:ws + BC].transpose(0, 2, 1).reshape(C, BC * P)
        m = dict(shared)
        m["attn_in"] = ap.reshape(BC * HEADS, P * JP)
        m["xT"] = np.ascontiguousarray(x_sl)
        in_maps.append(m)
    return nc, in_maps


def kernel(**inputs):
    nc, in_maps = prepare(**inputs)
    import os
    trace = bool(os.environ.get("BASS_TRACE"))
    if trace:
        _install_ntff_hook()
    res = run_bass_kernel_spmd(
        nc, in_maps, core_ids=list(range(NCORES)),
        trace=trace, tmpdir=os.environ.get("BASS_TRACE_DIR") or None,
    )
    global LAST_EXEC_NS
    LAST_EXEC_NS = res.exec_time_ns
    if res.exec_time_ns is not None:
        print(f"HW exec time: {res.exec_time_ns} ns")

    attn_new = np.empty((512, HEADS, P, P), np.float32)
    out = np.empty((4, C, P, P), np.float32)
    for k in range(NCORES):
        r = res.results[k]
        an = np.concatenate(
            [r["attn_new0"], r["attn_new1"]], axis=0
        ).astype(np.float32).reshape(BC, HEADS, P, JP)
        attn_new[k * BC:(k + 1) * BC] = an[:, :, :, 0:P]
        b_k = k // 2
        ws = (k % 2) * BC
        out[b_k, :, :, ws:ws + BC] = r["outT"].astype(np.float32).transpose(2, 1, 0)
    return out, attn_new


def _install_ntff_hook():
    import types
    try:
        import antenv.axon_hooks  # noqa: F401
        return
    except ImportError:
        pass
    try:
        from trn_agent_boot.trn_boot import _ntff_profile_via_ctypes
    except ImportError:
        return
    import antenv
    mod = types.ModuleType("antenv.axon_hooks")
    state = {"hook": None}
    mod.set_axon_ntff_profile_hook = lambda h: state.__setitem__("hook", h)
    mod.get_axon_ntff_profile_hook = lambda: state["hook"]
    sys.modules["antenv.axon_hooks"] = mod
    antenv.axon_hooks = mod
    hook = _ntff_profile_via_ctypes("/opt/axon/libaxon_pjrt.so")
    if hook is not None:
        mod.set_axon_ntff_profile_hook(hook)
